# revision 20
# baseline (speedup 1.0000x reference)
"""Llama GQA attention layer (T=2048, H=4096, 32 q heads / 8 kv heads, hd=128),
tensor-parallel over heads across 8 Trainium2 NeuronCores.

Per core c: 4 q heads + 1 kv head (wq/wk/wv column slices, wo row slice).
Each core computes a full [T, H] partial o_proj output; partials are summed on
host (the all-reduce of the TP scheme).

Device layout trick: host feeds hiddenT [H, T] so every matmul contracts over
the partition dim. Attention scores are produced transposed (k on partitions),
so softmax normalization uses a ones-vector matmul for the denominator and the
probabilities feed the PV matmul directly as the moving operand. All matmul
operands are float32r (fp32 storage, ~1e-4 matmul precision, 4x the fp32 PE
throughput at N>=256).
"""

import sys

if "/opt/trn_rl_repo" not in sys.path:
    sys.path.insert(0, "/opt/trn_rl_repo")

import numpy as np

import concourse.bass as bass
import concourse.bacc as bacc
import concourse.tile as tile
import concourse.mybir as mybir
from concourse import bass_utils

T = 2048
H = 4096
NQ = 32
NKV = 8
HD = 128
THETA = 10000.0
N_CORES = 8
NH = NQ // N_CORES          # local q heads per core
HALF = HD // 2
TC = 512                    # t-chunk (matmul free dim)
NTC = T // TC               # 4
NKCH = H // 128             # 32 hidden chunks
SCALE = float(HD) ** -0.5

F32 = mybir.dt.float32
F32R = mybir.dt.float32r
ALU = mybir.AluOpType
ACTF = mybir.ActivationFunctionType


def _build():
    nc = bacc.Bacc("TRN2", target_bir_lowering=False, debug=False,
                   num_devices=N_CORES)
    ht = nc.dram_tensor("ht", [NTC * NKCH, 128, TC], F32R,
                        kind="ExternalInput").ap()
    wq = nc.dram_tensor("wq", [H, NH * HD], F32R, kind="ExternalInput").ap()
    wk = nc.dram_tensor("wk", [H, HD], F32R, kind="ExternalInput").ap()
    wv = nc.dram_tensor("wv", [H, HD], F32R, kind="ExternalInput").ap()
    wo = nc.dram_tensor("wo", [NH * HD, H], F32R, kind="ExternalInput").ap()
    cos2 = nc.dram_tensor("cos2", [HD, T], F32, kind="ExternalInput").ap()
    sin2 = nc.dram_tensor("sin2", [HD, T], F32, kind="ExternalInput").ap()
    ident = nc.dram_tensor("ident", [128, 128], F32R, kind="ExternalInput").ap()
    ones = nc.dram_tensor("ones", [128, 1], F32R, kind="ExternalInput").ap()
    ones_r = nc.dram_tensor("ones_r", [1, 128], F32R, kind="ExternalInput").ap()
    masks = nc.dram_tensor("masks", [128, 4 * TC], F32, kind="ExternalInput").ap()
    out_t = nc.dram_tensor("out_t", [(H // 128) * NTC, 128, TC], F32,
                           kind="ExternalOutput").ap()

    with tile.TileContext(nc) as tc:
        _body(tc, ht, wq, wk, wv, wo, cos2, sin2, ident, ones, ones_r, masks, out_t)
    nc.compile()
    return nc


def _norm(nc, ps_rb, onesr_sb, attn_q, qc, rcs):
    """attn_q[qc] *= 1/den, broadcasting [1,TC] across partitions via a
    rank-1 PE matmul (ones_col x recip_row) into PSUM."""
    for h, rcr in enumerate(rcs):
        rb = ps_rb.tile([128, TC], F32, tag="rbps", name="rbps")
        nc.tensor.matmul(rb[:], onesr_sb[:], rcr[:], start=True, stop=True)
        sl = attn_q[qc][:, h * TC:(h + 1) * TC]
        nc.vector.tensor_tensor(sl, sl, rb[:], ALU.mult)


def _oproj(nc, ps_op, att, wo_t, attn_q, out_t, qc):
    """out_t[mo, t] = sum_h wo[f, mo] * attnT[f, t] for t-chunk qc."""
    for mo in range(H // 128):
        op = ps_op.tile([128, TC], F32, tag="op", name="op")
        for h in range(NH):
            nc.tensor.matmul(
                op[:],
                wo_t[h][:, mo * 128:(mo + 1) * 128],
                attn_q[qc][:, h * TC:(h + 1) * TC],
                start=(h == 0), stop=(h == NH - 1))
        ob = att.tile([128, TC], F32, tag="ob", name="ob")
        nc.scalar.copy(ob[:], op[:])
        nc.sync.dma_start(out_t[mo * NTC + qc], ob[:])


def _body(tc, ht, wq, wk, wv, wo, cos2, sin2, ident, ones, ones_r, masks, out_t):
    nc = tc.nc

    with (
        tc.tile_pool(name="persist", bufs=1) as persist,
        tc.tile_pool(name="small", bufs=1) as small,
    ):
        # live across all phases
        qT = persist.tile([128, NH * T], F32R, tag="qT")        # [d, h*T + t]
        kT = persist.tile([128, T], F32R, tag="kT")             # [d, t]
        v_sb = persist.tile([128, T], F32R, tag="v")            # chunk i cols: v[t=i*128+p, d]
        ident_sb = small.tile([128, 128], F32R, tag="ident")
        ones_sb = small.tile([128, 1], F32R, tag="ones")
        onesr_sb = small.tile([1, 128], F32R, tag="onesr")
        mask_sb = small.tile([128, 4 * TC], F32, tag="mask")    # 4 diag 0/1 masks
        nc.sync.dma_start(ident_sb[:], ident[:, :])
        nc.sync.dma_start(ones_sb[:], ones[:, :])
        nc.sync.dma_start(onesr_sb[:], ones_r[:, :])

        # ---------------- phase 1: QKV projections + RoPE + V transpose ----
        with (
            tc.tile_pool(name="ph1w", bufs=1) as ph1w,
            tc.tile_pool(name="ph1", bufs=6) as ph1,
            tc.tile_pool(name="rope", bufs=2) as rope,
            tc.tile_pool(name="ps1", bufs=1, space="PSUM") as ps1,
            tc.tile_pool(name="pst", bufs=2, space="PSUM") as pst,
        ):
            cos_sb = ph1w.tile([128, T], F32, tag="cos")
            sin_sb = ph1w.tile([128, T], F32, tag="sin")
            vT_sb = ph1w.tile([128, T], F32R, tag="vT")
            wq_t, wk_t, wv_t = [], [], []

            for t in range(NTC):
                qps = [ps1.tile([128, TC], F32, tag=f"qps{fc}", name=f"qps{fc}")
                       for fc in range(NH)]
                kps = ps1.tile([128, TC], F32, tag="kps")
                vps = ps1.tile([128, TC], F32, tag="vps")
                for k in range(NKCH):
                    if t == 0:
                        # stream weights chunk-by-chunk so MMs start immediately
                        wkt = ph1w.tile([128, 128], F32R, tag=f"wk{k}",
                                        name=f"wk{k}")
                        wvt = ph1w.tile([128, 128], F32R, tag=f"wv{k}",
                                        name=f"wv{k}")
                        wqt = ph1w.tile([128, 512], F32R, tag=f"wq{k}",
                                        name=f"wq{k}")
                        nc.sync.dma_start(wkt[:], wk[k * 128:(k + 1) * 128, :])
                        nc.sync.dma_start(wvt[:], wv[k * 128:(k + 1) * 128, :])
                        nc.sync.dma_start(wqt[:], wq[k * 128:(k + 1) * 128, :])
                        wq_t.append(wqt)
                        wk_t.append(wkt)
                        wv_t.append(wvt)
                    if t == 0 and k == 4:
                        nc.sync.dma_start(cos_sb[:], cos2[:, :])
                        nc.sync.dma_start(sin_sb[:], sin2[:, :])
                    if t == 1 and k == 0:
                        nc.sync.dma_start(mask_sb[:], masks[:, :])
                    htt = ph1.tile([128, TC], F32R, tag="ht")
                    nc.sync.dma_start(htt[:], ht[t * NKCH + k])
                    st, sp = (k == 0), (k == NKCH - 1)
                    nc.tensor.matmul(kps[:], wk_t[k][:], htt[:], start=st, stop=sp)
                    nc.tensor.matmul(vps[:], wv_t[k][:], htt[:], start=st, stop=sp)
                    for fc in range(NH):
                        nc.tensor.matmul(
                            qps[fc][:],
                            wq_t[k][:, fc * 128:(fc + 1) * 128],
                            htt[:], start=st, stop=sp)

                # RoPE: out = x*cos2 + swap(x)*sin2   (swap = halves exchanged)
                for hc in range(NH + 1):
                    src = qps[hc] if hc < NH else kps
                    dst = (qT[:, hc * T + t * TC: hc * T + (t + 1) * TC]
                           if hc < NH else kT[:, t * TC:(t + 1) * TC])
                    raw = rope.tile([128, TC], F32, tag="raw")
                    nc.scalar.copy(raw[:], src[:])   # frees the PSUM bank
                    sw = rope.tile([128, TC], F32, tag="sw")
                    nc.sync.dma_start(sw[0:HALF, :], raw[HALF:128, :])
                    nc.sync.dma_start(sw[HALF:128, :], raw[0:HALF, :])
                    a = rope.tile([128, TC], F32, tag="ra")
                    b = rope.tile([128, TC], F32, tag="rb")
                    nc.vector.tensor_tensor(
                        a[:], raw[:], cos_sb[:, t * TC:(t + 1) * TC], ALU.mult)
                    nc.vector.tensor_tensor(
                        b[:], sw[:], sin_sb[:, t * TC:(t + 1) * TC], ALU.mult)
                    nc.vector.tensor_tensor(dst, a[:], b[:], ALU.add)
                # V has no rope; stash vT, transpose below
                nc.scalar.copy(vT_sb[:, t * TC:(t + 1) * TC], vps[:])

            # transpose vT -> v (natural [t, d] layout, chunked along free dim)
            for i in range(T // 128):
                tp = pst.tile([128, 128], F32R, tag="tp")
                nc.tensor.transpose(tp[:], vT_sb[:, i * 128:(i + 1) * 128],
                                    ident_sb[:])
                nc.scalar.copy(v_sb[:, i * 128:(i + 1) * 128], tp[:])

        # ------- phase 2+3: attention interleaved with o_proj, per q-chunk --
        with (
            tc.tile_pool(name="late", bufs=1) as late,
            tc.tile_pool(name="att", bufs=4) as att,
            tc.tile_pool(name="rbp", bufs=4) as rbp,
            tc.tile_pool(name="ps_s", bufs=2, space="PSUM") as ps_s,
            tc.tile_pool(name="ps_o", bufs=2, space="PSUM") as ps_o,
            tc.tile_pool(name="ps_d", bufs=1, space="PSUM") as ps_d,
            tc.tile_pool(name="ps_op", bufs=2, space="PSUM") as ps_op,
            tc.tile_pool(name="ps_rb", bufs=1, space="PSUM") as ps_rb,
        ):
            attn_q = [late.tile([128, NH * TC], F32R, tag=f"attnq{i}",
                                name=f"attnq{i}") for i in range(NTC)]
            wo_t = []
            for h in range(NH):
                wot = late.tile([128, H], F32R, tag=f"wo{h}", name=f"wo{h}")
                nc.sync.dma_start(wot[:], wo[h * 128:(h + 1) * 128, :])
                wo_t.append(wot)

            norm_pending = []
            for qc in range(NTC):
                nkc = (qc + 1) * (TC // 128)     # causal k chunks of 128
                rcs = []
                for h in range(NH):
                    qh = qT[:, h * T + qc * TC: h * T + (qc + 1) * TC]
                    po = ps_o.tile([128, TC], F32, tag="po")
                    pd = ps_d.tile([1, TC], F32, tag="pd")
                    ps = []

                    def _pv(kc):
                        st, sp = (kc == 0), (kc == nkc - 1)
                        nc.tensor.matmul(po[:], v_sb[:, kc * 128:(kc + 1) * 128],
                                         ps[kc][:], start=st, stop=sp,
                                         skip_group_check=True)
                        nc.tensor.matmul(pd[:], ones_sb[:], ps[kc][:],
                                         start=st, stop=sp,
                                         skip_group_check=True)

                    for kc in range(nkc):
                        sT = ps_s.tile([128, TC], F32, tag="sT")
                        nc.tensor.matmul(sT[:], kT[:, kc * 128:(kc + 1) * 128],
                                         qh, start=True, stop=True)
                        p = att.tile([128, TC], F32R, tag="p", name="p")
                        nc.scalar.activation(p[:], sT[:], ACTF.Exp, scale=SCALE)
                        di = kc - (nkc - 4)
                        if di >= 0:
                            # zero the upper-triangle part of the diagonal tile
                            nc.vector.tensor_tensor(
                                p[:], p[:],
                                mask_sb[:, di * TC:(di + 1) * TC], ALU.mult)
                        ps.append(p)
                        # PV trails the score stream by 2 chunks so the PE
                        # never waits on the exp
                        if kc >= 2:
                            _pv(kc - 2)
                    for kc in range(max(0, nkc - 2), nkc):
                        _pv(kc)
                    # drain PSUM fast; normalization happens in _norm, off
                    # the PE critical path
                    nc.vector.tensor_copy(
                        attn_q[qc][:, h * TC:(h + 1) * TC], po[:])
                    rc = rbp.tile([1, TC], F32, tag="rc")
                    nc.vector.reciprocal_approx_fast(out=rc[:], in_=pd[:])
                    rcr = rbp.tile([1, TC], F32R, tag="rcr")
                    nc.scalar.copy(rcr[:], rc[:])
                    rcs.append(rcr)
                norm_pending.append((qc, rcs))

                # normalize + o_proj pipelined one q-chunk behind attention so
                # the reciprocal chain never stalls the PE stream
                if qc >= 1:
                    _norm(nc, ps_rb, onesr_sb, attn_q, *norm_pending.pop(0))
                    _oproj(nc, ps_op, att, wo_t, attn_q, out_t, qc - 1)
            _norm(nc, ps_rb, onesr_sb, attn_q, *norm_pending.pop(0))
            _oproj(nc, ps_op, att, wo_t, attn_q, out_t, NTC - 1)


_NC = None
LAST_EXEC_NS = None
LAST_TRACE = None


def _ensure_profile_hook():
    """Register the axon NTFF profiling hook (container lacks antenv.axon_hooks)."""
    import types
    import antenv
    if "antenv.axon_hooks" in sys.modules:
        return
    hooks_mod = types.ModuleType("antenv.axon_hooks")
    _h = [None]
    hooks_mod.set_axon_ntff_profile_hook = lambda hk: _h.__setitem__(0, hk)
    hooks_mod.get_axon_ntff_profile_hook = lambda: _h[0]
    sys.modules["antenv.axon_hooks"] = hooks_mod
    antenv.axon_hooks = hooks_mod
    from trn_agent_boot.trn_boot import _ntff_profile_via_ctypes
    hooks_mod.set_axon_ntff_profile_hook(
        _ntff_profile_via_ctypes("/opt/axon/libaxon_pjrt.so"))
    bass_utils.upload_artifacts = lambda tmpdir: "local://skipped"


def kernel(positions, hidden_states, wq, wk, wv, wo, _trace=False, **_unused):
    global _NC, LAST_EXEC_NS, LAST_TRACE
    positions = np.asarray(positions)
    hidden_states = np.asarray(hidden_states, dtype=np.float32)
    wq = np.asarray(wq, dtype=np.float32)
    wk = np.asarray(wk, dtype=np.float32)
    wv = np.asarray(wv, dtype=np.float32)
    wo = np.asarray(wo, dtype=np.float32)

    # host-side input prep (sharding + layout)
    hT = hidden_states.T                                            # [H, T]
    # tile-contiguous layout [(tc*32+k), 128, 512]: every DMA is one
    # sequential 256KB HBM read
    ht_tiles = np.ascontiguousarray(
        hT.reshape(NKCH, 128, NTC, TC).transpose(2, 0, 1, 3)
    ).reshape(NTC * NKCH, 128, TC)
    inv_freq = (1.0 / (THETA ** (np.arange(HALF, dtype=np.float64) / HALF)))
    ang = positions.astype(np.float64)[:, None] * inv_freq[None, :]  # [T, 64]
    cos = np.cos(ang).astype(np.float32).T                           # [64, T]
    sin = np.sin(ang).astype(np.float32).T
    cos2 = np.ascontiguousarray(np.concatenate([cos, cos], axis=0))  # [128, T]
    sin2 = np.ascontiguousarray(np.concatenate([-sin, sin], axis=0))
    ident = np.eye(128, dtype=np.float32)
    # diagonal-tile causal masks: masks_np[dk, di*TC+dq] = 1 if dq >= dk+128*di
    dk = np.arange(128)[:, None]
    dq = np.arange(TC)[None, :]
    masks_np = np.concatenate(
        [(dq >= dk + 128 * di).astype(np.float32) for di in range(4)], axis=1)
    masks_np = np.ascontiguousarray(masks_np)

    in_maps = []
    for c in range(N_CORES):
        in_maps.append({
            "ht": ht_tiles,
            "wq": np.ascontiguousarray(wq[:, c * NH * HD:(c + 1) * NH * HD]),
            "wk": np.ascontiguousarray(wk[:, c * HD:(c + 1) * HD]),
            "wv": np.ascontiguousarray(wv[:, c * HD:(c + 1) * HD]),
            "wo": np.ascontiguousarray(wo[c * NH * HD:(c + 1) * NH * HD, :]),
            "cos2": cos2,
            "sin2": sin2,
            "ident": ident,
            "ones": np.ones((128, 1), dtype=np.float32),
            "ones_r": np.ones((1, 128), dtype=np.float32),
            "masks": masks_np,
        })

    if _NC is None:
        _NC = _build()
    if _trace:
        _ensure_profile_hook()
    res = bass_utils.run_bass_kernel_spmd(
        _NC, in_maps, core_ids=list(range(N_CORES)), trace=_trace)
    if _trace:
        LAST_EXEC_NS = res.exec_time_ns
        LAST_TRACE = (res.instructions_and_trace[1]
                      if res.instructions_and_trace else None)

    acc = res.results[0]["out_t"].astype(np.float64)
    for c in range(1, N_CORES):
        acc += res.results[c]["out_t"]
    # [(mo*NTC+qc), 128, 512] -> [H, T] -> [T, H]
    out_ht = acc.reshape(H // 128, NTC, 128, TC).transpose(0, 2, 1, 3).reshape(H, T)
    return np.ascontiguousarray(out_ht.T).astype(np.float32)


# revision 21
# speedup vs baseline: 1.0079x; 1.0079x over previous
"""Llama GQA attention layer (T=2048, H=4096, 32 q heads / 8 kv heads, hd=128),
tensor-parallel over heads across 8 Trainium2 NeuronCores.

Per core c: 4 q heads + 1 kv head (wq/wk/wv column slices, wo row slice).
Each core computes a full [T, H] partial o_proj output; partials are summed on
host (the all-reduce of the TP scheme).

Device layout trick: host feeds hiddenT [H, T] so every matmul contracts over
the partition dim. Attention scores are produced transposed (k on partitions),
so softmax normalization uses a ones-vector matmul for the denominator and the
probabilities feed the PV matmul directly as the moving operand. All matmul
operands are float32r (fp32 storage, ~1e-4 matmul precision, 4x the fp32 PE
throughput at N>=256).
"""

import sys

if "/opt/trn_rl_repo" not in sys.path:
    sys.path.insert(0, "/opt/trn_rl_repo")

import numpy as np

import concourse.bass as bass
import concourse.bacc as bacc
import concourse.tile as tile
import concourse.mybir as mybir
from concourse import bass_utils

T = 2048
H = 4096
NQ = 32
NKV = 8
HD = 128
THETA = 10000.0
N_CORES = 8
NH = NQ // N_CORES          # local q heads per core
HALF = HD // 2
TC = 512                    # t-chunk (matmul free dim)
NTC = T // TC               # 4
NKCH = H // 128             # 32 hidden chunks
SCALE = float(HD) ** -0.5

F32 = mybir.dt.float32
F32R = mybir.dt.float32r
ALU = mybir.AluOpType
ACTF = mybir.ActivationFunctionType


def _build():
    nc = bacc.Bacc("TRN2", target_bir_lowering=False, debug=False,
                   num_devices=N_CORES)
    ht = nc.dram_tensor("ht", [NTC * NKCH, 128, TC], F32R,
                        kind="ExternalInput").ap()
    wq = nc.dram_tensor("wq", [H, NH * HD], F32R, kind="ExternalInput").ap()
    wk = nc.dram_tensor("wk", [H, HD], F32R, kind="ExternalInput").ap()
    wv = nc.dram_tensor("wv", [H, HD], F32R, kind="ExternalInput").ap()
    wo = nc.dram_tensor("wo", [NH * HD, H], F32R, kind="ExternalInput").ap()
    cos2 = nc.dram_tensor("cos2", [HD, T], F32, kind="ExternalInput").ap()
    sin2 = nc.dram_tensor("sin2", [HD, T], F32, kind="ExternalInput").ap()
    ident = nc.dram_tensor("ident", [128, 128], F32R, kind="ExternalInput").ap()
    ones = nc.dram_tensor("ones", [128, 1], F32R, kind="ExternalInput").ap()
    ones_r = nc.dram_tensor("ones_r", [1, 128], F32R, kind="ExternalInput").ap()
    masks = nc.dram_tensor("masks", [128, 4 * TC], F32, kind="ExternalInput").ap()
    out_t = nc.dram_tensor("out_t", [(H // 128) * NTC, 128, TC], F32,
                           kind="ExternalOutput").ap()

    with tile.TileContext(nc) as tc:
        _body(tc, ht, wq, wk, wv, wo, cos2, sin2, ident, ones, ones_r, masks, out_t)
    nc.compile()
    return nc


def _norm(nc, ps_rb, onesr_sb, attn_q, qc, rcs):
    """attn_q[qc] *= 1/den, broadcasting [1,TC] across partitions via a
    rank-1 PE matmul (ones_col x recip_row) into PSUM."""
    for h, rcr in enumerate(rcs):
        rb = ps_rb.tile([128, TC], F32, tag="rbps", name="rbps")
        nc.tensor.matmul(rb[:], onesr_sb[:], rcr[:], start=True, stop=True)
        sl = attn_q[qc][:, h * TC:(h + 1) * TC]
        nc.vector.tensor_tensor(sl, sl, rb[:], ALU.mult)


def _oproj(nc, ps_op, att, wo_t, attn_q, out_t, qc):
    """out_t[mo, t] = sum_h wo[f, mo] * attnT[f, t] for t-chunk qc."""
    for mo in range(H // 128):
        op = ps_op.tile([128, TC], F32, tag="op", name="op")
        for h in range(NH):
            nc.tensor.matmul(
                op[:],
                wo_t[h][:, mo * 128:(mo + 1) * 128],
                attn_q[qc][:, h * TC:(h + 1) * TC],
                start=(h == 0), stop=(h == NH - 1))
        ob = att.tile([128, TC], F32, tag="ob", name="ob")
        nc.scalar.copy(ob[:], op[:])
        nc.sync.dma_start(out_t[mo * NTC + qc], ob[:])


def _body(tc, ht, wq, wk, wv, wo, cos2, sin2, ident, ones, ones_r, masks, out_t):
    nc = tc.nc

    with (
        tc.tile_pool(name="persist", bufs=1) as persist,
        tc.tile_pool(name="small", bufs=1) as small,
    ):
        # live across all phases
        qT = persist.tile([128, NH * T], F32R, tag="qT")        # [d, h*T + t]
        kT = persist.tile([128, T], F32R, tag="kT")             # [d, t]
        v_sb = persist.tile([128, T], F32R, tag="v")            # chunk i cols: v[t=i*128+p, d]
        ident_sb = small.tile([128, 128], F32R, tag="ident")
        ones_sb = small.tile([128, 1], F32R, tag="ones")
        onesr_sb = small.tile([1, 128], F32R, tag="onesr")
        mask_sb = small.tile([128, 4 * TC], F32, tag="mask")    # 4 diag 0/1 masks
        nc.sync.dma_start(ident_sb[:], ident[:, :])
        nc.sync.dma_start(ones_sb[:], ones[:, :])
        nc.sync.dma_start(onesr_sb[:], ones_r[:, :])

        # ---------------- phase 1: QKV projections + RoPE + V transpose ----
        with (
            tc.tile_pool(name="ph1w", bufs=1) as ph1w,
            tc.tile_pool(name="ph1", bufs=6) as ph1,
            tc.tile_pool(name="rope", bufs=2) as rope,
            tc.tile_pool(name="ps1", bufs=1, space="PSUM") as ps1,
            tc.tile_pool(name="pst", bufs=2, space="PSUM") as pst,
        ):
            cos_sb = ph1w.tile([128, T], F32, tag="cos")
            sin_sb = ph1w.tile([128, T], F32, tag="sin")
            vT_sb = ph1w.tile([128, T], F32R, tag="vT")
            wq_t, wk_t, wv_t = [], [], []

            for t in range(NTC):
                qps = [ps1.tile([128, TC], F32, tag=f"qps{fc}", name=f"qps{fc}")
                       for fc in range(NH)]
                kps = ps1.tile([128, TC], F32, tag="kps")
                vps = ps1.tile([128, TC], F32, tag="vps")
                for k in range(NKCH):
                    if t == 0:
                        # stream weights chunk-by-chunk so MMs start immediately
                        wkt = ph1w.tile([128, 128], F32R, tag=f"wk{k}",
                                        name=f"wk{k}")
                        wvt = ph1w.tile([128, 128], F32R, tag=f"wv{k}",
                                        name=f"wv{k}")
                        wqt = ph1w.tile([128, 512], F32R, tag=f"wq{k}",
                                        name=f"wq{k}")
                        nc.sync.dma_start(wkt[:], wk[k * 128:(k + 1) * 128, :])
                        nc.sync.dma_start(wvt[:], wv[k * 128:(k + 1) * 128, :])
                        nc.sync.dma_start(wqt[:], wq[k * 128:(k + 1) * 128, :])
                        wq_t.append(wqt)
                        wk_t.append(wkt)
                        wv_t.append(wvt)
                    if t == 0 and k == 4:
                        nc.sync.dma_start(cos_sb[:], cos2[:, :])
                        nc.sync.dma_start(sin_sb[:], sin2[:, :])
                    if t == 1 and k == 0:
                        nc.sync.dma_start(mask_sb[:], masks[:, :])
                    htt = ph1.tile([128, TC], F32R, tag="ht")
                    nc.sync.dma_start(htt[:], ht[t * NKCH + k])
                    st, sp = (k == 0), (k == NKCH - 1)
                    for fc in range(NH):
                        nc.tensor.matmul(
                            qps[fc][:],
                            wq_t[k][:, fc * 128:(fc + 1) * 128],
                            htt[:], start=st, stop=sp)
                    nc.tensor.matmul(kps[:], wk_t[k][:], htt[:], start=st, stop=sp)
                    nc.tensor.matmul(vps[:], wv_t[k][:], htt[:], start=st, stop=sp)

                # RoPE: out = x*cos2 + swap(x)*sin2   (swap = halves exchanged)
                for hc in range(NH + 1):
                    src = qps[hc] if hc < NH else kps
                    dst = (qT[:, hc * T + t * TC: hc * T + (t + 1) * TC]
                           if hc < NH else kT[:, t * TC:(t + 1) * TC])
                    raw = rope.tile([128, TC], F32, tag="raw")
                    nc.scalar.copy(raw[:], src[:])   # frees the PSUM bank
                    sw = rope.tile([128, TC], F32, tag="sw")
                    nc.sync.dma_start(sw[0:HALF, :], raw[HALF:128, :])
                    nc.sync.dma_start(sw[HALF:128, :], raw[0:HALF, :])
                    a = rope.tile([128, TC], F32, tag="ra")
                    b = rope.tile([128, TC], F32, tag="rb")
                    nc.vector.tensor_tensor(
                        a[:], raw[:], cos_sb[:, t * TC:(t + 1) * TC], ALU.mult)
                    nc.vector.tensor_tensor(
                        b[:], sw[:], sin_sb[:, t * TC:(t + 1) * TC], ALU.mult)
                    nc.vector.tensor_tensor(dst, a[:], b[:], ALU.add)
                # V has no rope; stash vT, transpose below
                nc.scalar.copy(vT_sb[:, t * TC:(t + 1) * TC], vps[:])

            # transpose vT -> v (natural [t, d] layout, chunked along free dim)
            for i in range(T // 128):
                tp = pst.tile([128, 128], F32R, tag="tp")
                nc.tensor.transpose(tp[:], vT_sb[:, i * 128:(i + 1) * 128],
                                    ident_sb[:])
                nc.scalar.copy(v_sb[:, i * 128:(i + 1) * 128], tp[:])

        # ------- phase 2+3: attention interleaved with o_proj, per q-chunk --
        with (
            tc.tile_pool(name="late", bufs=1) as late,
            tc.tile_pool(name="att", bufs=4) as att,
            tc.tile_pool(name="rbp", bufs=4) as rbp,
            tc.tile_pool(name="ps_s", bufs=2, space="PSUM") as ps_s,
            tc.tile_pool(name="ps_o", bufs=2, space="PSUM") as ps_o,
            tc.tile_pool(name="ps_d", bufs=1, space="PSUM") as ps_d,
            tc.tile_pool(name="ps_op", bufs=2, space="PSUM") as ps_op,
            tc.tile_pool(name="ps_rb", bufs=1, space="PSUM") as ps_rb,
        ):
            attn_q = [late.tile([128, NH * TC], F32R, tag=f"attnq{i}",
                                name=f"attnq{i}") for i in range(NTC)]
            wo_t = []
            for h in range(NH):
                wot = late.tile([128, H], F32R, tag=f"wo{h}", name=f"wo{h}")
                nc.sync.dma_start(wot[:], wo[h * 128:(h + 1) * 128, :])
                wo_t.append(wot)

            norm_pending = []
            for qc in range(NTC):
                nkc = (qc + 1) * (TC // 128)     # causal k chunks of 128
                rcs = []
                for h in range(NH):
                    qh = qT[:, h * T + qc * TC: h * T + (qc + 1) * TC]
                    po = ps_o.tile([128, TC], F32, tag="po")
                    pd = ps_d.tile([1, TC], F32, tag="pd")
                    ps = []

                    def _pv(kc):
                        st, sp = (kc == 0), (kc == nkc - 1)
                        nc.tensor.matmul(po[:], v_sb[:, kc * 128:(kc + 1) * 128],
                                         ps[kc][:], start=st, stop=sp,
                                         skip_group_check=True)
                        nc.tensor.matmul(pd[:], ones_sb[:], ps[kc][:],
                                         start=st, stop=sp,
                                         skip_group_check=True)

                    for kc in range(nkc):
                        sT = ps_s.tile([128, TC], F32, tag="sT")
                        nc.tensor.matmul(sT[:], kT[:, kc * 128:(kc + 1) * 128],
                                         qh, start=True, stop=True)
                        p = att.tile([128, TC], F32R, tag="p", name="p")
                        nc.scalar.activation(p[:], sT[:], ACTF.Exp, scale=SCALE)
                        di = kc - (nkc - 4)
                        if di >= 0:
                            # zero the upper-triangle part of the diagonal tile
                            nc.vector.tensor_tensor(
                                p[:], p[:],
                                mask_sb[:, di * TC:(di + 1) * TC], ALU.mult)
                        ps.append(p)
                        # PV trails the score stream by 2 chunks so the PE
                        # never waits on the exp
                        if kc >= 2:
                            _pv(kc - 2)
                    for kc in range(max(0, nkc - 2), nkc):
                        _pv(kc)
                    # drain PSUM fast; normalization happens in _norm, off
                    # the PE critical path
                    nc.vector.tensor_copy(
                        attn_q[qc][:, h * TC:(h + 1) * TC], po[:])
                    rc = rbp.tile([1, TC], F32, tag="rc")
                    nc.vector.reciprocal_approx_fast(out=rc[:], in_=pd[:])
                    rcr = rbp.tile([1, TC], F32R, tag="rcr")
                    nc.scalar.copy(rcr[:], rc[:])
                    rcs.append(rcr)
                norm_pending.append((qc, rcs))

                # normalize + o_proj pipelined one q-chunk behind attention so
                # the reciprocal chain never stalls the PE stream
                if qc >= 1:
                    _norm(nc, ps_rb, onesr_sb, attn_q, *norm_pending.pop(0))
                    _oproj(nc, ps_op, att, wo_t, attn_q, out_t, qc - 1)
            _norm(nc, ps_rb, onesr_sb, attn_q, *norm_pending.pop(0))
            _oproj(nc, ps_op, att, wo_t, attn_q, out_t, NTC - 1)


_NC = None
LAST_EXEC_NS = None
LAST_TRACE = None


def _ensure_profile_hook():
    """Register the axon NTFF profiling hook (container lacks antenv.axon_hooks)."""
    import types
    import antenv
    if "antenv.axon_hooks" in sys.modules:
        return
    hooks_mod = types.ModuleType("antenv.axon_hooks")
    _h = [None]
    hooks_mod.set_axon_ntff_profile_hook = lambda hk: _h.__setitem__(0, hk)
    hooks_mod.get_axon_ntff_profile_hook = lambda: _h[0]
    sys.modules["antenv.axon_hooks"] = hooks_mod
    antenv.axon_hooks = hooks_mod
    from trn_agent_boot.trn_boot import _ntff_profile_via_ctypes
    hooks_mod.set_axon_ntff_profile_hook(
        _ntff_profile_via_ctypes("/opt/axon/libaxon_pjrt.so"))
    bass_utils.upload_artifacts = lambda tmpdir: "local://skipped"


def kernel(positions, hidden_states, wq, wk, wv, wo, _trace=False, **_unused):
    global _NC, LAST_EXEC_NS, LAST_TRACE
    positions = np.asarray(positions)
    hidden_states = np.asarray(hidden_states, dtype=np.float32)
    wq = np.asarray(wq, dtype=np.float32)
    wk = np.asarray(wk, dtype=np.float32)
    wv = np.asarray(wv, dtype=np.float32)
    wo = np.asarray(wo, dtype=np.float32)

    # host-side input prep (sharding + layout)
    hT = hidden_states.T                                            # [H, T]
    # tile-contiguous layout [(tc*32+k), 128, 512]: every DMA is one
    # sequential 256KB HBM read
    ht_tiles = np.ascontiguousarray(
        hT.reshape(NKCH, 128, NTC, TC).transpose(2, 0, 1, 3)
    ).reshape(NTC * NKCH, 128, TC)
    inv_freq = (1.0 / (THETA ** (np.arange(HALF, dtype=np.float64) / HALF)))
    ang = positions.astype(np.float64)[:, None] * inv_freq[None, :]  # [T, 64]
    cos = np.cos(ang).astype(np.float32).T                           # [64, T]
    sin = np.sin(ang).astype(np.float32).T
    cos2 = np.ascontiguousarray(np.concatenate([cos, cos], axis=0))  # [128, T]
    sin2 = np.ascontiguousarray(np.concatenate([-sin, sin], axis=0))
    ident = np.eye(128, dtype=np.float32)
    # diagonal-tile causal masks: masks_np[dk, di*TC+dq] = 1 if dq >= dk+128*di
    dk = np.arange(128)[:, None]
    dq = np.arange(TC)[None, :]
    masks_np = np.concatenate(
        [(dq >= dk + 128 * di).astype(np.float32) for di in range(4)], axis=1)
    masks_np = np.ascontiguousarray(masks_np)

    in_maps = []
    for c in range(N_CORES):
        in_maps.append({
            "ht": ht_tiles,
            "wq": np.ascontiguousarray(wq[:, c * NH * HD:(c + 1) * NH * HD]),
            "wk": np.ascontiguousarray(wk[:, c * HD:(c + 1) * HD]),
            "wv": np.ascontiguousarray(wv[:, c * HD:(c + 1) * HD]),
            "wo": np.ascontiguousarray(wo[c * NH * HD:(c + 1) * NH * HD, :]),
            "cos2": cos2,
            "sin2": sin2,
            "ident": ident,
            "ones": np.ones((128, 1), dtype=np.float32),
            "ones_r": np.ones((1, 128), dtype=np.float32),
            "masks": masks_np,
        })

    if _NC is None:
        _NC = _build()
    if _trace:
        _ensure_profile_hook()
    res = bass_utils.run_bass_kernel_spmd(
        _NC, in_maps, core_ids=list(range(N_CORES)), trace=_trace)
    if _trace:
        LAST_EXEC_NS = res.exec_time_ns
        LAST_TRACE = (res.instructions_and_trace[1]
                      if res.instructions_and_trace else None)

    acc = res.results[0]["out_t"].astype(np.float64)
    for c in range(1, N_CORES):
        acc += res.results[c]["out_t"]
    # [(mo*NTC+qc), 128, 512] -> [H, T] -> [T, H]
    out_ht = acc.reshape(H // 128, NTC, 128, TC).transpose(0, 2, 1, 3).reshape(H, T)
    return np.ascontiguousarray(out_ht.T).astype(np.float32)


# revision 22
# speedup vs baseline: 1.0182x; 1.0102x over previous
"""Llama GQA attention layer (T=2048, H=4096, 32 q heads / 8 kv heads, hd=128),
tensor-parallel over heads across 8 Trainium2 NeuronCores.

Per core c: 4 q heads + 1 kv head (wq/wk/wv column slices, wo row slice).
Each core computes a full [T, H] partial o_proj output; partials are summed on
host (the all-reduce of the TP scheme).

Device layout trick: host feeds hiddenT [H, T] so every matmul contracts over
the partition dim. Attention scores are produced transposed (k on partitions),
so softmax normalization uses a ones-vector matmul for the denominator and the
probabilities feed the PV matmul directly as the moving operand. All matmul
operands are float32r (fp32 storage, ~1e-4 matmul precision, 4x the fp32 PE
throughput at N>=256).
"""

import sys

if "/opt/trn_rl_repo" not in sys.path:
    sys.path.insert(0, "/opt/trn_rl_repo")

import numpy as np

import concourse.bass as bass
import concourse.bacc as bacc
import concourse.tile as tile
import concourse.mybir as mybir
from concourse import bass_utils

T = 2048
H = 4096
NQ = 32
NKV = 8
HD = 128
THETA = 10000.0
N_CORES = 8
NH = NQ // N_CORES          # local q heads per core
HALF = HD // 2
TC = 512                    # t-chunk (matmul free dim)
NTC = T // TC               # 4
NKCH = H // 128             # 32 hidden chunks
SCALE = float(HD) ** -0.5

F32 = mybir.dt.float32
F32R = mybir.dt.float32r
ALU = mybir.AluOpType
ACTF = mybir.ActivationFunctionType


def _build():
    nc = bacc.Bacc("TRN2", target_bir_lowering=False, debug=False,
                   num_devices=N_CORES)
    ht = nc.dram_tensor("ht", [NTC * NKCH, 128, TC], F32R,
                        kind="ExternalInput").ap()
    wq = nc.dram_tensor("wq", [H, NH * HD], F32R, kind="ExternalInput").ap()
    wk = nc.dram_tensor("wk", [H, HD], F32R, kind="ExternalInput").ap()
    wv = nc.dram_tensor("wv", [H, HD], F32R, kind="ExternalInput").ap()
    wo = nc.dram_tensor("wo", [NH * HD, H], F32R, kind="ExternalInput").ap()
    cos2 = nc.dram_tensor("cos2", [HD, T], F32, kind="ExternalInput").ap()
    sin2 = nc.dram_tensor("sin2", [HD, T], F32, kind="ExternalInput").ap()
    ident = nc.dram_tensor("ident", [128, 128], F32R, kind="ExternalInput").ap()
    ones = nc.dram_tensor("ones", [128, 1], F32R, kind="ExternalInput").ap()
    ones_r = nc.dram_tensor("ones_r", [1, 128], F32R, kind="ExternalInput").ap()
    masks = nc.dram_tensor("masks", [128, 4 * TC], F32, kind="ExternalInput").ap()
    out_t = nc.dram_tensor("out_t", [(H // 128) * NTC, 128, TC], F32,
                           kind="ExternalOutput").ap()

    with tile.TileContext(nc) as tc:
        _body(tc, ht, wq, wk, wv, wo, cos2, sin2, ident, ones, ones_r, masks, out_t)
    nc.compile()
    return nc


def _norm(nc, ps_rb, onesr_sb, attn_q, qc, rcs):
    """attn_q[qc] *= 1/den, broadcasting [1,TC] across partitions via a
    rank-1 PE matmul (ones_col x recip_row) into PSUM."""
    for h, rcr in enumerate(rcs):
        rb = ps_rb.tile([128, TC], F32, tag="rbps", name="rbps")
        nc.tensor.matmul(rb[:], onesr_sb[:], rcr[:], start=True, stop=True)
        sl = attn_q[qc][:, h * TC:(h + 1) * TC]
        nc.vector.tensor_tensor(sl, sl, rb[:], ALU.mult)


def _oproj(nc, ps_op, att, wo_t, attn_q, out_t, qc):
    """out_t[mo, t] = sum_h wo[f, mo] * attnT[f, t] for t-chunk qc."""
    for mo in range(H // 128):
        op = ps_op.tile([128, TC], F32, tag="op", name="op")
        for h in range(NH):
            nc.tensor.matmul(
                op[:],
                wo_t[h][:, mo * 128:(mo + 1) * 128],
                attn_q[qc][:, h * TC:(h + 1) * TC],
                start=(h == 0), stop=(h == NH - 1))
        ob = att.tile([128, TC], F32, tag="ob", name="ob")
        nc.scalar.copy(ob[:], op[:])
        nc.sync.dma_start(out_t[mo * NTC + qc], ob[:])


def _body(tc, ht, wq, wk, wv, wo, cos2, sin2, ident, ones, ones_r, masks, out_t):
    nc = tc.nc

    with (
        tc.tile_pool(name="persist", bufs=1) as persist,
        tc.tile_pool(name="small", bufs=1) as small,
    ):
        # live across all phases
        qT = persist.tile([128, NH * T], F32R, tag="qT")        # [d, h*T + t]
        kT = persist.tile([128, T], F32R, tag="kT")             # [d, t]
        v_sb = persist.tile([128, T], F32R, tag="v")            # chunk i cols: v[t=i*128+p, d]
        ident_sb = small.tile([128, 128], F32R, tag="ident")
        ones_sb = small.tile([128, 1], F32R, tag="ones")
        onesr_sb = small.tile([1, 128], F32R, tag="onesr")
        mask_sb = small.tile([128, 4 * TC], F32, tag="mask")    # 4 diag 0/1 masks
        nc.sync.dma_start(ident_sb[:], ident[:, :])
        nc.sync.dma_start(ones_sb[:], ones[:, :])
        nc.sync.dma_start(onesr_sb[:], ones_r[:, :])

        # ---------------- phase 1: QKV projections + RoPE + V transpose ----
        with (
            tc.tile_pool(name="ph1w", bufs=1) as ph1w,
            tc.tile_pool(name="ph1", bufs=6) as ph1,
            tc.tile_pool(name="rope", bufs=2) as rope,
            tc.tile_pool(name="ps1", bufs=1, space="PSUM") as ps1,
            tc.tile_pool(name="pst", bufs=2, space="PSUM") as pst,
        ):
            cos_sb = ph1w.tile([128, T], F32, tag="cos")
            sin_sb = ph1w.tile([128, T], F32, tag="sin")
            vT_sb = ph1w.tile([128, T], F32R, tag="vT")
            wq_t, wk_t, wv_t = [], [], []

            for t in range(NTC):
                qps = [ps1.tile([128, TC], F32, tag=f"qps{fc}", name=f"qps{fc}")
                       for fc in range(NH)]
                kps = ps1.tile([128, TC], F32, tag="kps")
                vps = ps1.tile([128, TC], F32, tag="vps")
                for k in range(NKCH):
                    if t == 0:
                        # stream weights chunk-by-chunk so MMs start immediately
                        wqt = ph1w.tile([128, 512], F32R, tag=f"wq{k}",
                                        name=f"wq{k}")
                        wkt = ph1w.tile([128, 128], F32R, tag=f"wk{k}",
                                        name=f"wk{k}")
                        wvt = ph1w.tile([128, 128], F32R, tag=f"wv{k}",
                                        name=f"wv{k}")
                        nc.sync.dma_start(wqt[:], wq[k * 128:(k + 1) * 128, :])
                        nc.sync.dma_start(wkt[:], wk[k * 128:(k + 1) * 128, :])
                        nc.sync.dma_start(wvt[:], wv[k * 128:(k + 1) * 128, :])
                        wq_t.append(wqt)
                        wk_t.append(wkt)
                        wv_t.append(wvt)
                    if t == 0 and k == 4:
                        nc.sync.dma_start(cos_sb[:], cos2[:, :])
                        nc.sync.dma_start(sin_sb[:], sin2[:, :])
                    if t == 1 and k == 0:
                        nc.sync.dma_start(mask_sb[:], masks[:, :])
                    htt = ph1.tile([128, TC], F32R, tag="ht")
                    nc.sync.dma_start(htt[:], ht[t * NKCH + k])
                    st, sp = (k == 0), (k == NKCH - 1)
                    for fc in range(NH):
                        nc.tensor.matmul(
                            qps[fc][:],
                            wq_t[k][:, fc * 128:(fc + 1) * 128],
                            htt[:], start=st, stop=sp)
                    nc.tensor.matmul(kps[:], wk_t[k][:], htt[:], start=st, stop=sp)
                    nc.tensor.matmul(vps[:], wv_t[k][:], htt[:], start=st, stop=sp)

                # RoPE: out = x*cos2 + swap(x)*sin2   (swap = halves exchanged)
                for hc in range(NH + 1):
                    src = qps[hc] if hc < NH else kps
                    dst = (qT[:, hc * T + t * TC: hc * T + (t + 1) * TC]
                           if hc < NH else kT[:, t * TC:(t + 1) * TC])
                    raw = rope.tile([128, TC], F32, tag="raw")
                    nc.scalar.copy(raw[:], src[:])   # frees the PSUM bank
                    sw = rope.tile([128, TC], F32, tag="sw")
                    nc.sync.dma_start(sw[0:HALF, :], raw[HALF:128, :])
                    nc.sync.dma_start(sw[HALF:128, :], raw[0:HALF, :])
                    a = rope.tile([128, TC], F32, tag="ra")
                    b = rope.tile([128, TC], F32, tag="rb")
                    nc.vector.tensor_tensor(
                        a[:], raw[:], cos_sb[:, t * TC:(t + 1) * TC], ALU.mult)
                    nc.vector.tensor_tensor(
                        b[:], sw[:], sin_sb[:, t * TC:(t + 1) * TC], ALU.mult)
                    nc.vector.tensor_tensor(dst, a[:], b[:], ALU.add)
                # V has no rope; stash vT, transpose below
                nc.scalar.copy(vT_sb[:, t * TC:(t + 1) * TC], vps[:])

            # transpose vT -> v (natural [t, d] layout, chunked along free dim)
            for i in range(T // 128):
                tp = pst.tile([128, 128], F32R, tag="tp")
                nc.tensor.transpose(tp[:], vT_sb[:, i * 128:(i + 1) * 128],
                                    ident_sb[:])
                nc.scalar.copy(v_sb[:, i * 128:(i + 1) * 128], tp[:])

        # ------- phase 2+3: attention interleaved with o_proj, per q-chunk --
        with (
            tc.tile_pool(name="late", bufs=1) as late,
            tc.tile_pool(name="att", bufs=4) as att,
            tc.tile_pool(name="rbp", bufs=4) as rbp,
            tc.tile_pool(name="ps_s", bufs=2, space="PSUM") as ps_s,
            tc.tile_pool(name="ps_o", bufs=2, space="PSUM") as ps_o,
            tc.tile_pool(name="ps_d", bufs=1, space="PSUM") as ps_d,
            tc.tile_pool(name="ps_op", bufs=2, space="PSUM") as ps_op,
            tc.tile_pool(name="ps_rb", bufs=1, space="PSUM") as ps_rb,
        ):
            attn_q = [late.tile([128, NH * TC], F32R, tag=f"attnq{i}",
                                name=f"attnq{i}") for i in range(NTC)]
            wo_t = []
            for h in range(NH):
                wot = late.tile([128, H], F32R, tag=f"wo{h}", name=f"wo{h}")
                nc.sync.dma_start(wot[:], wo[h * 128:(h + 1) * 128, :])
                wo_t.append(wot)

            norm_pending = []
            for qc in range(NTC):
                nkc = (qc + 1) * (TC // 128)     # causal k chunks of 128
                rcs = []
                for h in range(NH):
                    qh = qT[:, h * T + qc * TC: h * T + (qc + 1) * TC]
                    po = ps_o.tile([128, TC], F32, tag="po")
                    pd = ps_d.tile([1, TC], F32, tag="pd")
                    ps = []

                    def _pv(kc):
                        st, sp = (kc == 0), (kc == nkc - 1)
                        nc.tensor.matmul(po[:], v_sb[:, kc * 128:(kc + 1) * 128],
                                         ps[kc][:], start=st, stop=sp,
                                         skip_group_check=True)
                        nc.tensor.matmul(pd[:], ones_sb[:], ps[kc][:],
                                         start=st, stop=sp,
                                         skip_group_check=True)

                    for kc in range(nkc):
                        sT = ps_s.tile([128, TC], F32, tag="sT")
                        nc.tensor.matmul(sT[:], kT[:, kc * 128:(kc + 1) * 128],
                                         qh, start=True, stop=True)
                        p = att.tile([128, TC], F32R, tag="p", name="p")
                        nc.scalar.activation(p[:], sT[:], ACTF.Exp, scale=SCALE)
                        di = kc - (nkc - 4)
                        if di >= 0:
                            # zero the upper-triangle part of the diagonal tile
                            nc.vector.tensor_tensor(
                                p[:], p[:],
                                mask_sb[:, di * TC:(di + 1) * TC], ALU.mult)
                        ps.append(p)
                        # PV trails the score stream by 2 chunks so the PE
                        # never waits on the exp
                        if kc >= 2:
                            _pv(kc - 2)
                    for kc in range(max(0, nkc - 2), nkc):
                        _pv(kc)
                    # drain PSUM fast; normalization happens in _norm, off
                    # the PE critical path
                    nc.vector.tensor_copy(
                        attn_q[qc][:, h * TC:(h + 1) * TC], po[:])
                    rc = rbp.tile([1, TC], F32, tag="rc")
                    nc.vector.reciprocal_approx_fast(out=rc[:], in_=pd[:])
                    rcr = rbp.tile([1, TC], F32R, tag="rcr")
                    nc.scalar.copy(rcr[:], rc[:])
                    rcs.append(rcr)
                norm_pending.append((qc, rcs))

                # normalize + o_proj pipelined one q-chunk behind attention so
                # the reciprocal chain never stalls the PE stream
                if qc >= 1:
                    _norm(nc, ps_rb, onesr_sb, attn_q, *norm_pending.pop(0))
                    _oproj(nc, ps_op, att, wo_t, attn_q, out_t, qc - 1)
            _norm(nc, ps_rb, onesr_sb, attn_q, *norm_pending.pop(0))
            _oproj(nc, ps_op, att, wo_t, attn_q, out_t, NTC - 1)


_NC = None
LAST_EXEC_NS = None
LAST_TRACE = None


def _ensure_profile_hook():
    """Register the axon NTFF profiling hook (container lacks antenv.axon_hooks)."""
    import types
    import antenv
    if "antenv.axon_hooks" in sys.modules:
        return
    hooks_mod = types.ModuleType("antenv.axon_hooks")
    _h = [None]
    hooks_mod.set_axon_ntff_profile_hook = lambda hk: _h.__setitem__(0, hk)
    hooks_mod.get_axon_ntff_profile_hook = lambda: _h[0]
    sys.modules["antenv.axon_hooks"] = hooks_mod
    antenv.axon_hooks = hooks_mod
    from trn_agent_boot.trn_boot import _ntff_profile_via_ctypes
    hooks_mod.set_axon_ntff_profile_hook(
        _ntff_profile_via_ctypes("/opt/axon/libaxon_pjrt.so"))
    bass_utils.upload_artifacts = lambda tmpdir: "local://skipped"


def kernel(positions, hidden_states, wq, wk, wv, wo, _trace=False, **_unused):
    global _NC, LAST_EXEC_NS, LAST_TRACE
    positions = np.asarray(positions)
    hidden_states = np.asarray(hidden_states, dtype=np.float32)
    wq = np.asarray(wq, dtype=np.float32)
    wk = np.asarray(wk, dtype=np.float32)
    wv = np.asarray(wv, dtype=np.float32)
    wo = np.asarray(wo, dtype=np.float32)

    # host-side input prep (sharding + layout)
    hT = hidden_states.T                                            # [H, T]
    # tile-contiguous layout [(tc*32+k), 128, 512]: every DMA is one
    # sequential 256KB HBM read
    ht_tiles = np.ascontiguousarray(
        hT.reshape(NKCH, 128, NTC, TC).transpose(2, 0, 1, 3)
    ).reshape(NTC * NKCH, 128, TC)
    inv_freq = (1.0 / (THETA ** (np.arange(HALF, dtype=np.float64) / HALF)))
    ang = positions.astype(np.float64)[:, None] * inv_freq[None, :]  # [T, 64]
    cos = np.cos(ang).astype(np.float32).T                           # [64, T]
    sin = np.sin(ang).astype(np.float32).T
    cos2 = np.ascontiguousarray(np.concatenate([cos, cos], axis=0))  # [128, T]
    sin2 = np.ascontiguousarray(np.concatenate([-sin, sin], axis=0))
    ident = np.eye(128, dtype=np.float32)
    # diagonal-tile causal masks: masks_np[dk, di*TC+dq] = 1 if dq >= dk+128*di
    dk = np.arange(128)[:, None]
    dq = np.arange(TC)[None, :]
    masks_np = np.concatenate(
        [(dq >= dk + 128 * di).astype(np.float32) for di in range(4)], axis=1)
    masks_np = np.ascontiguousarray(masks_np)

    in_maps = []
    for c in range(N_CORES):
        in_maps.append({
            "ht": ht_tiles,
            "wq": np.ascontiguousarray(wq[:, c * NH * HD:(c + 1) * NH * HD]),
            "wk": np.ascontiguousarray(wk[:, c * HD:(c + 1) * HD]),
            "wv": np.ascontiguousarray(wv[:, c * HD:(c + 1) * HD]),
            "wo": np.ascontiguousarray(wo[c * NH * HD:(c + 1) * NH * HD, :]),
            "cos2": cos2,
            "sin2": sin2,
            "ident": ident,
            "ones": np.ones((128, 1), dtype=np.float32),
            "ones_r": np.ones((1, 128), dtype=np.float32),
            "masks": masks_np,
        })

    if _NC is None:
        _NC = _build()
    if _trace:
        _ensure_profile_hook()
    res = bass_utils.run_bass_kernel_spmd(
        _NC, in_maps, core_ids=list(range(N_CORES)), trace=_trace)
    if _trace:
        LAST_EXEC_NS = res.exec_time_ns
        LAST_TRACE = (res.instructions_and_trace[1]
                      if res.instructions_and_trace else None)

    acc = res.results[0]["out_t"].astype(np.float64)
    for c in range(1, N_CORES):
        acc += res.results[c]["out_t"]
    # [(mo*NTC+qc), 128, 512] -> [H, T] -> [T, H]
    out_ht = acc.reshape(H // 128, NTC, 128, TC).transpose(0, 2, 1, 3).reshape(H, T)
    return np.ascontiguousarray(out_ht.T).astype(np.float32)
